# revision 39
# baseline (speedup 1.0000x reference)
"""Trainium2 Bass kernel for nn_Correlation (stereo cost volume).

  out[b, d, h, w] = mean_c( x[b,c,h,w] * y[b,c,h,w-d] ),  w >= d else 0
  B=8, C=32, H=256, W=512, D=48  (maxdisp=48)

Sharding: data-parallel over batch B across the 8 NeuronCores (one batch
element per core).  Each core computes its full [D, H, W] cost volume.

Per-core algorithm (bf16 matmul, fp16 band output):
  - The host pre-scales x by 1/C, casts to bf16, and PRE-PERMUTES both
    inputs into the SBUF staging layout [128, ...] with partition
    (hb,c) = row 64*hb+g of channel c (y rows carry their 47-col lead
    inline, zero-padded at h=0).  Every DMA is then a plain 2-dim
    (partition x contiguous-run) pattern — the fast "direct 2D" HWDGE
    path — and input HBM traffic is halved vs fp32.
  - 4 h-rows {g, 64+g, 128+g, 192+g} are packed per matmul via a
    BLOCK-DIAGONAL stationary: K = 4*C = 128 (the PE HAM clock-gates
    K=32 matmuls to half clock) and the moving window per 32-wide
    w-subtile is 79 cols: 1264 streamed cols per 4 rows vs 2800 for
    K=32.
  - Stationary tiles (x interleaved (st, hb, j), zero off-diagonal
    blocks) are built by four 512-col DVE copies per group; the zero
    blocks are memset ONCE per buffer (4-ring) and never dirtied.
  - 16 matmuls per group write bank-packed PSUM ([128,79] at col
    offsets 0..395 inside [128,474]/[128,316] tiles); ACT (2.5) and
    DVE (0.5) drain them to fp16 band tiles; one [128,2528] store per
    PAIR of groups.
  - One DMA instruction only engages a 4-engine group (~100 GB/s), so
    slab loads are split into 4x 32-partition chunks and PREFETCHED a
    full slab ahead of the stores on the in-order sync HWDGE ring
    (16 engines, ~400 GB/s, no head-of-line blocking on drain sems).
  - The HOST extracts the 48 diagonals (band col = st*79 + j+47-d at
    partition hb*32+j, h = 64*hb + g) with one as_strided view, casts
    to fp32, and zeroes the invalid w < d region.
"""

import sys

sys.path.insert(0, "/opt/trn_rl_repo")

import numpy as np
import ml_dtypes
from contextlib import ExitStack

import concourse.bass as bass
import concourse.tile as tile
from concourse import mybir
from concourse import bass_utils

B = 8
C = 32
H = 256
W = 512
D = 48
LEAD = D - 1            # 47
HB = 4                  # h rows packed per group (K = HB*C = 128)
NG = H // HB            # 64 groups; group g = rows {g, 64+g, 128+g, 192+g}
HS = H // HB            # 64: h-stride between the packed rows
ST = 16                 # 32-col w-subtiles per row
SW = W // ST            # 32 subtile width
MN = SW + LEAD          # 79 moving cols per subtile matmul
PACK = 6                # psum tiles packed per PSUM bank (6*79*4B < 2KB)
PACK_W = PACK * MN      # 474 band cols per psum bank
GBW = 3 * PACK_W        # 1422 stored band cols per group (158 junk)
QS = 8                  # groups staged per input slab
NSLAB = NG // QS        # 8 slabs
EXT = PACK_W - 4 * MN   # 158 extra y cols read by the widened st=15
XROW = HS * W           # 32768 x cols per staged partition row
YROW = LEAD + HS * W + EXT   # 32973 y cols per staged partition row
XW = QS * W             # 4096 staged x cols per slab
YW = LEAD + QS * W + EXT     # 4301 staged y cols per slab


def _split_waits(nc, max_waits=1):
    """Walrus codegen accepts at most ONE sync wait per instruction; Tile
    attaches several.  Split extra waits onto preceding NoOps on the same
    engine queue (dispatch is in-order, waits gate dispatch)."""
    for fn in nc.m.functions:
        for blk in fn.blocks:
            newl = []
            changed = False
            for inst in blk.instructions:
                si = getattr(inst, "sync_info", None)
                ow = list(si.on_wait) if si is not None and si.on_wait else []
                if len(ow) > max_waits and inst.engine is not None:
                    for k, wcond in enumerate(ow[:-max_waits]):
                        newl.append(mybir.InstNoOp(
                            name=f"{inst.name}w{k}",
                            engine=inst.engine,
                            sync_info=mybir.SyncInfo(on_wait=[wcond],
                                                     on_update=[]),
                        ))
                    inst.sync_info = mybir.SyncInfo(
                        on_wait=ow[-max_waits:],
                        on_update=list(si.on_update) if si.on_update else [])
                    changed = True
                newl.append(inst)
            if changed:
                blk.instructions = newl


def _emit_body(ctx, tc, x_ap, y_ap, o_ap):
    nc = tc.nc
    o_t = o_ap.tensor
    x_t = x_ap.tensor
    y_t = y_ap.tensor

    xspool = ctx.enter_context(tc.tile_pool(name="xs", bufs=1))
    xppool = ctx.enter_context(tc.tile_pool(name="xp", bufs=3))
    ypool = ctx.enter_context(tc.tile_pool(name="yp", bufs=3))
    gpool = ctx.enter_context(tc.tile_pool(name="gp", bufs=10))
    ppool = ctx.enter_context(tc.tile_pool(name="pp", bufs=2, space="PSUM"))

    def load_slab(q, nchunk=1):
        """Chunked 2-dim loads for x and y of slab q (fast DGE path);
        chunks land on different 4-engine DMA groups.  x triggers ride
        the sync ring (stores dispatch promptly there), y triggers the
        scalar ring (ahead of the drains in program order)."""
        xp = xppool.tile([128, XW], mybir.dt.bfloat16, name=f"xp{q}",
                         tag="xp")
        yt = ypool.tile([128, YW], mybir.dt.bfloat16, name=f"yt{q}",
                        tag="yt")
        npart = 128 // nchunk
        for m in range(nchunk):
            nc.scalar.dma_start(
                xp[npart * m:npart * (m + 1), :],
                bass.AP(x_t, npart * m * XROW + q * XW,
                        [[XROW, npart], [1, XW]]))
            nc.scalar.dma_start(
                yt[npart * m:npart * (m + 1), :],
                bass.AP(y_t, npart * m * YROW + q * XW,
                        [[YROW, npart], [1, YW]]))
        return xp, yt

    # persistent stationary ring; zero blocks memset once, never dirtied
    NXS = 4
    xst = [xspool.tile([128, ST * 128], mybir.dt.bfloat16,
                       name=f"xst{i}", tag=f"xst{i}") for i in range(NXS)]

    # slab 0: small leading chunks covering just the first two groups so
    # compute starts ~8us earlier, then the bulk
    xp = xppool.tile([128, XW], mybir.dt.bfloat16, name="xp0", tag="xp")
    yt = ypool.tile([128, YW], mybir.dt.bfloat16, name="yt0", tag="yt")
    FW = 2 * W
    FYW = LEAD + 2 * W + EXT
    nc.scalar.dma_start(xp[:, 0:FW],
                        bass.AP(x_t, 0, [[XROW, 128], [1, FW]]))
    nc.scalar.dma_start(yt[:, 0:FYW],
                        bass.AP(y_t, 0, [[YROW, 128], [1, FYW]]))
    nc.scalar.dma_start(xp[:, FW:],
                        bass.AP(x_t, FW, [[XROW, 128], [1, XW - FW]]))
    nc.scalar.dma_start(yt[:, FYW:],
                        bass.AP(y_t, FYW, [[YROW, 128], [1, YW - FYW]]))
    for i in range(NXS):
        if i % 2 == 0:
            nc.vector.memset(xst[i][:, :], 0.0)
        else:
            nc.scalar.copy(xst[i][:, :], xst[i - 1][:, :])

    nxt1 = load_slab(1)                 # slab 1 prefetch
    for q in range(NSLAB):
        if q + 2 < NSLAB:
            nxt2 = load_slab(q + 2)     # prefetch TWO slabs ahead
        xp_t = xp[:, :].tensor

        for gq in range(QS):            # groups within the slab
            g = QS * q + gq             # global group index
            sg = xst[g % NXS]
            sg_t = sg[:, :].tensor
            gt = gpool.tile([128, GBW], mybir.dt.float16,
                            name=f"gt{g}", tag="gt")

            # ---- interleave into block-diagonal stationary ----
            # sg[p=(hb,c), st*128+hb*32+j] = xp[p, gq*512 + st*32+j]
            # (3 copies on DVE, 1 on GPSIMD)
            for hb in range(HB):
                dst = bass.AP(sg_t, hb * C * (ST * 128) + hb * SW,
                              [[ST * 128, C], [128, ST], [1, SW]])
                srcb = bass.AP(xp_t, hb * C * XW + gq * W,
                               [[XW, C], [SW, ST], [1, SW]])
                nc.vector.tensor_scalar_mul(dst, srcb, 1.0)

            # ---- 16 subtile matmuls into one 3-bank psum tile ----
            # st=15 widened to fill bank 2 through col 474, so the fused
            # drain below never reads unwritten psum
            pt = ppool.tile([128, 3 * 512], mybir.dt.float32,
                            name=f"ps{g}", tag="ps",
                            padded_shape=[128, 3 * 512])
            pt_t = pt[:, :].tensor
            for st in range(ST):
                ti, off = divmod(st, PACK)
                n = MN if st < ST - 1 else MN + (PACK_W - 4 * MN)
                nc.tensor.matmul(
                    pt[:, ti * 512 + off * MN: ti * 512 + off * MN + n],
                    sg[:, st * 128:(st + 1) * 128],
                    yt[:, gq * W + st * SW: gq * W + st * SW + n],
                    start=True, stop=True)

            # ---- fused single-instr band drain (ACT, 3 banks) ----
            nc.scalar.copy(
                bass.AP(gt[:, :].tensor, 0,
                        [[GBW, 128], [PACK_W, 3], [1, PACK_W]]),
                bass.AP(pt_t, 0, [[3 * 512, 128], [512, 3], [1, PACK_W]]))

            # ---- per-group 2-dim band store (sync HWDGE ring); the
            # last groups go out in 4 parallel chunks to shorten the tail
            if g < NG - 2:
                dst = bass.AP(o_t, g * 128 * GBW, [[GBW, 128], [1, GBW]])
                nc.sync.dma_start(dst, gt[:, :])
            else:
                for m in range(4):
                    dst = bass.AP(o_t, g * 128 * GBW + 32 * m * GBW,
                                  [[GBW, 32], [1, GBW]])
                    nc.sync.dma_start(dst, gt[32 * m:32 * (m + 1), :])

        if q + 1 < NSLAB:
            xp, yt = nxt1
        if q + 2 < NSLAB:
            nxt1 = nxt2


def _build_kernel():
    nc = bass.Bass(trn_type="TRN2", target_bir_lowering=False)
    x_d = nc.dram_tensor("x", [128, XROW], mybir.dt.bfloat16,
                         kind="ExternalInput")
    y_d = nc.dram_tensor("y", [128, YROW], mybir.dt.bfloat16,
                         kind="ExternalInput")
    o_d = nc.dram_tensor("o", [NG, 128, GBW], mybir.dt.float16,
                         kind="ExternalOutput")
    with ExitStack() as ctx:
        tc = ctx.enter_context(tile.TileContext(nc))
        _emit_body(ctx, tc, x_d.ap(), y_d.ap(), o_d.ap())
    _split_waits(nc)
    return nc


_NC_CACHE = None


def _get_nc():
    global _NC_CACHE
    if _NC_CACHE is None:
        _NC_CACHE = _build_kernel()
    return _NC_CACHE


def _prep_inputs(x: np.ndarray, y: np.ndarray):
    """Cast to bf16 (1/C folded into x — exact exponent shift) and
    pre-permute into the staged SBUF layouts:
      x2[b, p=(hb,c), g*512+w]       = (x/C)[b, c, 64*hb+g, w]
      y2[b, p=(hb,c), 47 + gg*512+w] = y[b, c, 64*hb+gg, w]  (lead inline)
    """
    xs = (np.asarray(x, dtype=np.float32) * np.float32(1.0 / C)).astype(
        ml_dtypes.bfloat16)
    ys = np.asarray(y, dtype=np.float32).astype(ml_dtypes.bfloat16)
    # x: [B, C, (hb, 64), W] -> [B, hb, C, 64*W]
    x2 = np.ascontiguousarray(
        xs.reshape(B, C, HB, HS, W).transpose(0, 2, 1, 3, 4)
        .reshape(B, 128, XROW))
    yf = ys.reshape(B, C, H * W)
    y2 = np.empty((B, HB, C, YROW), dtype=ml_dtypes.bfloat16)
    for hb in range(HB):
        s = hb * HS * W
        e = s + HS * W
        y2[:, hb, :, LEAD:LEAD + HS * W] = yf[:, :, s:e]
        if hb == 0:
            y2[:, 0, :, :LEAD] = ml_dtypes.bfloat16(0.0)
        else:
            y2[:, hb, :, :LEAD] = yf[:, :, s - LEAD:s]
        if hb == HB - 1:
            y2[:, hb, :, LEAD + HS * W:] = ml_dtypes.bfloat16(0.0)
        else:
            y2[:, hb, :, LEAD + HS * W:] = yf[:, :, e:e + EXT]
    y2 = np.ascontiguousarray(y2.reshape(B, 128, YROW))
    return x2, y2


def _deskew(band: np.ndarray) -> np.ndarray:
    """band: [NG, 128, GBW] fp16 -> [D, H, W] fp32 (w<d left unmasked)."""
    el = band.strides[-1]
    assert band.flags["C_CONTIGUOUS"]
    # view[g, hb, j, ti, off, d] =
    #     band[g, hb*32+j, ti*PACK_W + off*MN + j + LEAD - d]
    # (st = ti*PACK + off; slots st=16,17 are junk, dropped below)
    view = np.lib.stride_tricks.as_strided(
        band[:, :, LEAD:],
        shape=(NG, HB, SW, 3, PACK, D),
        strides=(band.strides[0], SW * band.strides[1], band.strides[1] + el,
                 PACK_W * el, MN * el, -el),
    )
    out = view.astype(np.float32)
    # h = 64*hb + g, w = (ti*PACK+off)*SW + j
    out = out.transpose(5, 1, 0, 3, 4, 2).reshape(D, H, 3 * PACK * SW)
    return out[:, :, :W]


def kernel(x: np.ndarray, y: np.ndarray, maxdisp=48) -> np.ndarray:
    assert int(maxdisp) == D
    x2, y2 = _prep_inputs(x, y)

    nc = _get_nc()
    in_maps = [{"x": x2[b], "y": y2[b]} for b in range(B)]
    res = bass_utils.run_bass_kernel_spmd(nc, in_maps, core_ids=list(range(B)))

    out = np.empty((B, D, H, W), dtype=np.float32)
    for b in range(B):
        band = np.asarray(res.results[b]["o"])
        out[b] = _deskew(band)
    # zero the invalid w < d zone
    for d in range(1, D):
        out[:, d, :, :d] = 0.0
    return out


if __name__ == "__main__":
    rng = np.random.default_rng(0)
    x = rng.standard_normal((B, C, H, W), dtype=np.float32)
    y = rng.standard_normal((B, C, H, W), dtype=np.float32)
    out = kernel(x=x, y=y, maxdisp=D)
    print("kernel output:", out.shape, out.dtype)


# revision 42
# speedup vs baseline: 1.0142x; 1.0142x over previous
"""Trainium2 Bass kernel for nn_Correlation (stereo cost volume).

  out[b, d, h, w] = mean_c( x[b,c,h,w] * y[b,c,h,w-d] ),  w >= d else 0
  B=8, C=32, H=256, W=512, D=48  (maxdisp=48)

Sharding: data-parallel over batch B across the 8 NeuronCores (one batch
element per core).  Each core computes its full [D, H, W] cost volume.

Per-core algorithm (bf16 matmul, fp16 band output):
  - The host pre-scales x by 1/C, casts to bf16, and PRE-PERMUTES both
    inputs into the SBUF staging layout [128, ...] with partition
    (hb,c) = row 64*hb+g of channel c (y rows carry their 47-col lead
    inline, zero-padded at h=0).  Every DMA is then a plain 2-dim
    (partition x contiguous-run) pattern — the fast "direct 2D" HWDGE
    path — and input HBM traffic is halved vs fp32.
  - 4 h-rows {g, 64+g, 128+g, 192+g} are packed per matmul via a
    BLOCK-DIAGONAL stationary: K = 4*C = 128 (the PE HAM clock-gates
    K=32 matmuls to half clock) and the moving window per 32-wide
    w-subtile is 79 cols: 1264 streamed cols per 4 rows vs 2800 for
    K=32.
  - Stationary tiles (x interleaved (st, hb, j), zero off-diagonal
    blocks) are built by four 512-col DVE copies per group; the zero
    blocks are memset ONCE per buffer (4-ring) and never dirtied.
  - 16 matmuls per group write ONE bank-packed 3-bank PSUM tile
    ([128,79] at col offsets ti*512 + off*79; the last matmul is
    widened to 237 cols so no unwritten psum is ever read); a SINGLE
    fused ACT copy ([[512,3],[1,474]]) drains all 1422 band cols
    (158 junk) to an fp16 tile per group.
  - DMA rings are specialized: input loads ride the scalar HWDGE ring
    (triggered a full slab ahead, before the drains in program order),
    band stores the sync ring.  One DMA instruction only engages a
    4-engine DMA group (~100 GB/s), so consecutive instructions
    rotating across groups provide the parallelism; deep gt buffering
    (10) hides the ~3.7us store latency.  Slab 0 is loaded
    small-chunks-first so compute starts ~8us earlier.
  - The HOST extracts the 48 diagonals (band col = (st//6)*474 +
    (st%6)*79 + j+47-d at partition hb*32+j, h = 64*hb + g) with one
    as_strided view, casts to fp32, and zeroes the invalid w < d zone.
"""

import sys

sys.path.insert(0, "/opt/trn_rl_repo")

import numpy as np
import ml_dtypes
from contextlib import ExitStack

import concourse.bass as bass
import concourse.tile as tile
from concourse import mybir
from concourse import bass_utils

B = 8
C = 32
H = 256
W = 512
D = 48
LEAD = D - 1            # 47
HB = 4                  # h rows packed per group (K = HB*C = 128)
NG = H // HB            # 64 groups; group g = rows {g, 64+g, 128+g, 192+g}
HS = H // HB            # 64: h-stride between the packed rows
ST = 16                 # 32-col w-subtiles per row
SW = W // ST            # 32 subtile width
MN = SW + LEAD          # 79 moving cols per subtile matmul
PACK = 6                # psum tiles packed per PSUM bank (6*79*4B < 2KB)
PACK_W = PACK * MN      # 474 band cols per psum bank
GBW = 3 * PACK_W        # 1422 stored band cols per group (158 junk)
QS = 8                  # groups staged per input slab
NSLAB = NG // QS        # 8 slabs
EXT = PACK_W - 4 * MN   # 158 extra y cols read by the widened st=15
XROW = HS * W           # 32768 x cols per staged partition row
YROW = LEAD + HS * W + EXT   # 32973 y cols per staged partition row
XW = QS * W             # 4096 staged x cols per slab
YW = LEAD + QS * W + EXT     # 4301 staged y cols per slab


def _split_waits(nc, max_waits=1):
    """Walrus codegen accepts at most ONE sync wait per instruction; Tile
    attaches several.  Split extra waits onto preceding NoOps on the same
    engine queue (dispatch is in-order, waits gate dispatch)."""
    for fn in nc.m.functions:
        for blk in fn.blocks:
            newl = []
            changed = False
            for inst in blk.instructions:
                si = getattr(inst, "sync_info", None)
                ow = list(si.on_wait) if si is not None and si.on_wait else []
                if len(ow) > max_waits and inst.engine is not None:
                    for k, wcond in enumerate(ow[:-max_waits]):
                        newl.append(mybir.InstNoOp(
                            name=f"{inst.name}w{k}",
                            engine=inst.engine,
                            sync_info=mybir.SyncInfo(on_wait=[wcond],
                                                     on_update=[]),
                        ))
                    inst.sync_info = mybir.SyncInfo(
                        on_wait=ow[-max_waits:],
                        on_update=list(si.on_update) if si.on_update else [])
                    changed = True
                newl.append(inst)
            if changed:
                blk.instructions = newl


def _emit_body(ctx, tc, x_ap, y_ap, o_ap):
    nc = tc.nc
    o_t = o_ap.tensor
    x_t = x_ap.tensor
    y_t = y_ap.tensor

    xspool = ctx.enter_context(tc.tile_pool(name="xs", bufs=1))
    xppool = ctx.enter_context(tc.tile_pool(name="xp", bufs=3))
    ypool = ctx.enter_context(tc.tile_pool(name="yp", bufs=3))
    gpool = ctx.enter_context(tc.tile_pool(name="gp", bufs=10))
    ppool = ctx.enter_context(tc.tile_pool(name="pp", bufs=2, space="PSUM"))

    def load_slab(q, nchunk=1):
        """Chunked 2-dim loads for x and y of slab q (fast DGE path);
        chunks land on different 4-engine DMA groups.  x triggers ride
        the sync ring (stores dispatch promptly there), y triggers the
        scalar ring (ahead of the drains in program order)."""
        xp = xppool.tile([128, XW], mybir.dt.bfloat16, name=f"xp{q}",
                         tag="xp")
        yt = ypool.tile([128, YW], mybir.dt.bfloat16, name=f"yt{q}",
                        tag="yt")
        npart = 128 // nchunk
        for m in range(nchunk):
            nc.scalar.dma_start(
                xp[npart * m:npart * (m + 1), :],
                bass.AP(x_t, npart * m * XROW + q * XW,
                        [[XROW, npart], [1, XW]]))
            nc.scalar.dma_start(
                yt[npart * m:npart * (m + 1), :],
                bass.AP(y_t, npart * m * YROW + q * XW,
                        [[YROW, npart], [1, YW]]))
        return xp, yt

    # persistent stationary ring; zero blocks memset once, never dirtied
    NXS = 4
    xst = [xspool.tile([128, ST * 128], mybir.dt.bfloat16,
                       name=f"xst{i}", tag=f"xst{i}") for i in range(NXS)]

    # slab 0: small leading chunks covering just the first two groups so
    # compute starts ~8us earlier, then the bulk
    xp = xppool.tile([128, XW], mybir.dt.bfloat16, name="xp0", tag="xp")
    yt = ypool.tile([128, YW], mybir.dt.bfloat16, name="yt0", tag="yt")
    FW = 2 * W
    FYW = LEAD + 2 * W + EXT
    nc.scalar.dma_start(xp[:, 0:FW],
                        bass.AP(x_t, 0, [[XROW, 128], [1, FW]]))
    nc.scalar.dma_start(yt[:, 0:FYW],
                        bass.AP(y_t, 0, [[YROW, 128], [1, FYW]]))
    nc.scalar.dma_start(xp[:, FW:],
                        bass.AP(x_t, FW, [[XROW, 128], [1, XW - FW]]))
    nc.scalar.dma_start(yt[:, FYW:],
                        bass.AP(y_t, FYW, [[YROW, 128], [1, YW - FYW]]))
    for i in range(NXS):
        if i % 2 == 0:
            nc.vector.memset(xst[i][:, :], 0.0)
        else:
            nc.scalar.copy(xst[i][:, :], xst[i - 1][:, :])

    for q in range(NSLAB):
        if q + 1 < NSLAB:
            nxt = load_slab(q + 1)      # prefetch a full slab ahead
        xp_t = xp[:, :].tensor

        for gq in range(QS):            # groups within the slab
            g = QS * q + gq             # global group index
            sg = xst[g % NXS]
            sg_t = sg[:, :].tensor
            gt = gpool.tile([128, GBW], mybir.dt.float16,
                            name=f"gt{g}", tag="gt")

            # ---- interleave into block-diagonal stationary ----
            # sg[p=(hb,c), st*128+hb*32+j] = xp[p, gq*512 + st*32+j]
            # (3 copies on DVE, 1 on GPSIMD)
            for hb in range(HB):
                dst = bass.AP(sg_t, hb * C * (ST * 128) + hb * SW,
                              [[ST * 128, C], [128, ST], [1, SW]])
                srcb = bass.AP(xp_t, hb * C * XW + gq * W,
                               [[XW, C], [SW, ST], [1, SW]])
                nc.vector.tensor_scalar_mul(dst, srcb, 1.0)

            # ---- 16 subtile matmuls into one 3-bank psum tile ----
            # st=15 widened to fill bank 2 through col 474, so the fused
            # drain below never reads unwritten psum
            pt = ppool.tile([128, 3 * 512], mybir.dt.float32,
                            name=f"ps{g}", tag="ps",
                            padded_shape=[128, 3 * 512])
            pt_t = pt[:, :].tensor
            for st in range(ST):
                ti, off = divmod(st, PACK)
                n = MN if st < ST - 1 else MN + (PACK_W - 4 * MN)
                nc.tensor.matmul(
                    pt[:, ti * 512 + off * MN: ti * 512 + off * MN + n],
                    sg[:, st * 128:(st + 1) * 128],
                    yt[:, gq * W + st * SW: gq * W + st * SW + n],
                    start=True, stop=True)

            # ---- fused single-instr band drain (ACT, 3 banks) ----
            nc.scalar.copy(
                bass.AP(gt[:, :].tensor, 0,
                        [[GBW, 128], [PACK_W, 3], [1, PACK_W]]),
                bass.AP(pt_t, 0, [[3 * 512, 128], [512, 3], [1, PACK_W]]))

            # ---- per-group 2-dim band store (sync HWDGE ring); the
            # last groups go out in 4 parallel chunks to shorten the tail
            if g < NG - 2:
                dst = bass.AP(o_t, g * 128 * GBW, [[GBW, 128], [1, GBW]])
                nc.sync.dma_start(dst, gt[:, :])
            else:
                for m in range(4):
                    dst = bass.AP(o_t, g * 128 * GBW + 32 * m * GBW,
                                  [[GBW, 32], [1, GBW]])
                    nc.sync.dma_start(dst, gt[32 * m:32 * (m + 1), :])

        if q + 1 < NSLAB:
            xp, yt = nxt


def _build_kernel():
    nc = bass.Bass(trn_type="TRN2", target_bir_lowering=False)
    x_d = nc.dram_tensor("x", [128, XROW], mybir.dt.bfloat16,
                         kind="ExternalInput")
    y_d = nc.dram_tensor("y", [128, YROW], mybir.dt.bfloat16,
                         kind="ExternalInput")
    o_d = nc.dram_tensor("o", [NG, 128, GBW], mybir.dt.float16,
                         kind="ExternalOutput")
    with ExitStack() as ctx:
        tc = ctx.enter_context(tile.TileContext(nc))
        _emit_body(ctx, tc, x_d.ap(), y_d.ap(), o_d.ap())
    _split_waits(nc)
    return nc


_NC_CACHE = None


def _get_nc():
    global _NC_CACHE
    if _NC_CACHE is None:
        _NC_CACHE = _build_kernel()
    return _NC_CACHE


def _prep_inputs(x: np.ndarray, y: np.ndarray):
    """Cast to bf16 (1/C folded into x — exact exponent shift) and
    pre-permute into the staged SBUF layouts:
      x2[b, p=(hb,c), g*512+w]       = (x/C)[b, c, 64*hb+g, w]
      y2[b, p=(hb,c), 47 + gg*512+w] = y[b, c, 64*hb+gg, w]  (lead inline)
    """
    xs = (np.asarray(x, dtype=np.float32) * np.float32(1.0 / C)).astype(
        ml_dtypes.bfloat16)
    ys = np.asarray(y, dtype=np.float32).astype(ml_dtypes.bfloat16)
    # x: [B, C, (hb, 64), W] -> [B, hb, C, 64*W]
    x2 = np.ascontiguousarray(
        xs.reshape(B, C, HB, HS, W).transpose(0, 2, 1, 3, 4)
        .reshape(B, 128, XROW))
    yf = ys.reshape(B, C, H * W)
    y2 = np.empty((B, HB, C, YROW), dtype=ml_dtypes.bfloat16)
    for hb in range(HB):
        s = hb * HS * W
        e = s + HS * W
        y2[:, hb, :, LEAD:LEAD + HS * W] = yf[:, :, s:e]
        if hb == 0:
            y2[:, 0, :, :LEAD] = ml_dtypes.bfloat16(0.0)
        else:
            y2[:, hb, :, :LEAD] = yf[:, :, s - LEAD:s]
        if hb == HB - 1:
            y2[:, hb, :, LEAD + HS * W:] = ml_dtypes.bfloat16(0.0)
        else:
            y2[:, hb, :, LEAD + HS * W:] = yf[:, :, e:e + EXT]
    y2 = np.ascontiguousarray(y2.reshape(B, 128, YROW))
    return x2, y2


def _deskew(band: np.ndarray) -> np.ndarray:
    """band: [NG, 128, GBW] fp16 -> [D, H, W] fp32 (w<d left unmasked)."""
    el = band.strides[-1]
    assert band.flags["C_CONTIGUOUS"]
    # view[g, hb, j, ti, off, d] =
    #     band[g, hb*32+j, ti*PACK_W + off*MN + j + LEAD - d]
    # (st = ti*PACK + off; slots st=16,17 are junk, dropped below)
    view = np.lib.stride_tricks.as_strided(
        band[:, :, LEAD:],
        shape=(NG, HB, SW, 3, PACK, D),
        strides=(band.strides[0], SW * band.strides[1], band.strides[1] + el,
                 PACK_W * el, MN * el, -el),
    )
    out = view.astype(np.float32)
    # h = 64*hb + g, w = (ti*PACK+off)*SW + j
    out = out.transpose(5, 1, 0, 3, 4, 2).reshape(D, H, 3 * PACK * SW)
    return out[:, :, :W]


def kernel(x: np.ndarray, y: np.ndarray, maxdisp=48) -> np.ndarray:
    assert int(maxdisp) == D
    x2, y2 = _prep_inputs(x, y)

    nc = _get_nc()
    in_maps = [{"x": x2[b], "y": y2[b]} for b in range(B)]
    res = bass_utils.run_bass_kernel_spmd(nc, in_maps, core_ids=list(range(B)))

    out = np.empty((B, D, H, W), dtype=np.float32)
    for b in range(B):
        band = np.asarray(res.results[b]["o"])
        out[b] = _deskew(band)
    # zero the invalid w < d zone
    for d in range(1, D):
        out[:, d, :, :d] = 0.0
    return out


if __name__ == "__main__":
    rng = np.random.default_rng(0)
    x = rng.standard_normal((B, C, H, W), dtype=np.float32)
    y = rng.standard_normal((B, C, H, W), dtype=np.float32)
    out = kernel(x=x, y=y, maxdisp=D)
    print("kernel output:", out.shape, out.dtype)
